# revision 1
# baseline (speedup 1.0000x reference)
"""Self-attention kernel for TRN2, data-parallel over batch (8 cores).

Per core (one batch element):
  q/k/v projections from xT (built via cast + TensorE transpose),
  scores computed TRANSPOSED (sT[s,t] blocks) so softmax exp feeds the
  PV matmul without transposing the 2048x2048 attention matrix,
  row sums via a ones-column appended to v (free), normalization folded
  into the output-projection epilogue (per-partition scalar), residual
  added in fp32.

Matmul inputs bf16, PSUM accumulation fp32, softmax/normalize/residual fp32.
"""

import numpy as np

import concourse.bass as bass
import concourse.mybir as mybir
import concourse.tile as tile
from concourse import bacc
from concourse.bass import ds, ts
from concourse.bass_utils import run_bass_kernel_spmd
from concourse.masks import make_identity

F32 = mybir.dt.float32
BF16 = mybir.dt.bfloat16
AF = mybir.ActivationFunctionType

B, T, C, U, P = 8, 2048, 512, 256, 128
TC = T // P   # 16 row tiles
CCH = C // P  # 4 c-chunks
UCH = U // P  # 2 u-chunks
TBLK = 512    # t-block for attention
NTB = T // TBLK
SCALE = 1.0 / float(np.sqrt(U))

_cache = {}


def _build_kernel(tc):
    nc = tc.nc
    x = nc.dram_tensor("x", [T, C], F32, kind="ExternalInput").ap()
    Wq = nc.dram_tensor("Wq", [C, U], F32, kind="ExternalInput").ap()
    bq = nc.dram_tensor("bq", [U], F32, kind="ExternalInput").ap()
    Wk = nc.dram_tensor("Wk", [C, U], F32, kind="ExternalInput").ap()
    bk = nc.dram_tensor("bk", [U], F32, kind="ExternalInput").ap()
    Wv = nc.dram_tensor("Wv", [C, U], F32, kind="ExternalInput").ap()
    bv = nc.dram_tensor("bv", [U], F32, kind="ExternalInput").ap()
    Wa = nc.dram_tensor("Wa", [U, C], F32, kind="ExternalInput").ap()
    ba = nc.dram_tensor("ba", [C], F32, kind="ExternalInput").ap()
    out = nc.dram_tensor("out", [T, C], F32, kind="ExternalOutput").ap()

    consts = tc.alloc_tile_pool(name="consts", bufs=1)
    persist = tc.alloc_tile_pool(name="persist", bufs=1)

    # --- constants / weights (bf16 via casting SWDGE DMA) ---
    ones_row = consts.tile([1, P], BF16)
    nc.vector.memset(ones_row, 1.0)
    identity = consts.tile([P, P], BF16)
    make_identity(nc, identity)
    Wq_bf = consts.tile([P, CCH, U], BF16)
    nc.gpsimd.dma_start(out=Wq_bf, in_=Wq.rearrange("(cc p) u -> p cc u", p=P))
    Wk_bf = consts.tile([P, CCH, U], BF16)
    nc.gpsimd.dma_start(out=Wk_bf, in_=Wk.rearrange("(cc p) u -> p cc u", p=P))
    Wv_bf = consts.tile([P, CCH, U], BF16)
    nc.gpsimd.dma_start(out=Wv_bf, in_=Wv.rearrange("(cc p) u -> p cc u", p=P))
    Wa_bf = consts.tile([P, UCH, C], BF16)
    nc.gpsimd.dma_start(out=Wa_bf, in_=Wa.rearrange("(uc p) c -> p uc c", p=P))
    bq_sb = consts.tile([P, UCH], F32)
    nc.sync.dma_start(out=bq_sb, in_=bq.rearrange("(uc p) -> p uc", p=P))
    bk_sb = consts.tile([P, UCH], F32)
    nc.sync.dma_start(out=bk_sb, in_=bk.rearrange("(uc p) -> p uc", p=P))
    bv_bf = consts.tile([1, U], BF16)
    nc.gpsimd.dma_start(out=bv_bf, in_=bv[None, :])
    ba_bf = consts.tile([1, C], BF16)
    nc.gpsimd.dma_start(out=ba_bf, in_=ba[None, :])

    # persistent layout tensors
    x_sb = persist.tile([P, TC, C], F32)      # x rows (residual + transpose src)
    xT_sb = persist.tile([P, CCH, T], BF16)   # x^T  (c on partitions)
    qT_sb = persist.tile([P, UCH, T], BF16)   # q^T  (u on partitions)
    kT_sb = persist.tile([P, UCH, T], BF16)   # k^T
    v_sb = persist.tile([P, TC, U + 1], BF16)  # v row-major + ones column
    aT_sb = persist.tile([P, UCH, T], BF16)   # a^T (unnormalized)
    nc.vector.memset(v_sb[:, :, U:U + 1], 1.0)

    with tc.tile_pool(name="warm", bufs=1, space="PSUM") as warm_pool:
        wtile = warm_pool.tile([P, P], F32, name="warmup")
        for i in range(36):
            nc.tensor.matmul(wtile, lhsT=identity, rhs=identity,
                             start=(i == 0), stop=(i == 35))

    for tt in range(TC):
        eng = nc.sync if tt % 2 == 0 else nc.scalar
        eng.dma_start(out=x_sb[:, tt, :], in_=x[ts(tt, P), :])

    # --- phase 1: xT via DVE cast + TensorE transpose ---
    with tc.tile_pool(name="xbf", bufs=4) as xbf_pool, \
         tc.tile_pool(name="tpsum", bufs=4, space="PSUM") as tpsum:
        for tt in range(TC):
            x_bf = xbf_pool.tile([P, C], BF16, tag="xbf")
            nc.vector.tensor_copy(out=x_bf, in_=x_sb[:, tt, :])
            for cc in range(CCH):
                tps = tpsum.tile([P, P], BF16, tag="tps")
                nc.tensor.transpose(tps, x_bf[:, ts(cc, P)], identity)
                nc.vector.tensor_copy(out=xT_sb[:, cc, ts(tt, P)], in_=tps)

    # --- phase 2: projections ---
    with tc.tile_pool(name="wpsum", bufs=2, space="PSUM") as wpsum, \
         tc.tile_pool(name="vpsum", bufs=2, space="PSUM") as vpsum:
        for (WT, bias_sb, dst) in ((Wq_bf, bq_sb, qT_sb), (Wk_bf, bk_sb, kT_sb)):
            for uc in range(UCH):
                for tb in range(NTB):
                    ps = wpsum.tile([P, TBLK], F32, tag="wps")
                    for cc in range(CCH):
                        nc.tensor.matmul(
                            ps,
                            lhsT=WT[:, cc, ts(uc, P)],
                            rhs=xT_sb[:, cc, ds(tb * TBLK, TBLK)],
                            start=(cc == 0),
                            stop=(cc == CCH - 1),
                        )
                    nc.scalar.activation(
                        out=dst[:, uc, ds(tb * TBLK, TBLK)],
                        in_=ps,
                        func=AF.Identity,
                        bias=bias_sb[:, uc:uc + 1],
                        scale=1.0,
                    )
        for tt in range(TC):
            ps = vpsum.tile([P, U], F32, tag="vps")
            for cc in range(CCH):
                nc.tensor.matmul(
                    ps,
                    lhsT=xT_sb[:, cc, ts(tt, P)],
                    rhs=Wv_bf[:, cc, :],
                    start=(cc == 0),
                    stop=False,
                )
            nc.tensor.matmul(ps, lhsT=ones_row, rhs=bv_bf, start=False, stop=True)
            nc.vector.tensor_copy(out=v_sb[:, tt, 0:U], in_=ps)

    # --- phase 3: attention per t-block ---
    spsum = tc.alloc_tile_pool(name="spsum", bufs=2, space="PSUM")
    apsum = tc.alloc_tile_pool(name="apsum", bufs=4, space="PSUM")
    ypsum = tc.alloc_tile_pool(name="ypsum", bufs=2, space="PSUM")
    p_pool = tc.alloc_tile_pool(name="p_pool", bufs=TC + 1)
    a_pool = tc.alloc_tile_pool(name="a_pool", bufs=4)
    rcp_pool = tc.alloc_tile_pool(name="rcp_pool", bufs=9)
    y_pool = tc.alloc_tile_pool(name="y_pool", bufs=3)

    deferred = [None]

    def finish(tb, rcps):
        for tsl in range(NTB):
            row0 = tb * TBLK + tsl * P
            yps = ypsum.tile([P, C], F32, tag="yps")
            for uc in range(UCH):
                nc.tensor.matmul(
                    yps,
                    lhsT=aT_sb[:, uc, ds(row0, P)],
                    rhs=Wa_bf[:, uc, :],
                    start=(uc == 0),
                    stop=False,
                )
            nc.tensor.matmul(yps, lhsT=ones_row, rhs=ba_bf, start=False, stop=True)
            y_sb = y_pool.tile([P, C], F32, tag="ysb")
            nc.vector.tensor_scalar(
                out=y_sb, in0=yps, scalar1=rcps[tsl], scalar2=None,
                op0=mybir.AluOpType.mult,
            )
            nc.vector.tensor_add(out=y_sb, in0=y_sb, in1=x_sb[:, tb * NTB + tsl, :])
            nc.sync.dma_start(out=out[ds(row0, P), :], in_=y_sb)

    def pv_col(sc, apss):
        for tsl in range(NTB):
            nc.tensor.matmul(
                apss[tsl],
                lhsT=pts[sc][:, ts(tsl, P)],
                rhs=v_sb[:, sc, :],
                start=(sc == 0),
                stop=(sc == TC - 1),
            )

    for tb in range(NTB):
        # scores (transposed) + exp, with PV trailing 2 stages behind
        pts = []
        apss = [apsum.tile([P, U + 1], F32, tag="aps", name=f"aps{tb}_{i}") for i in range(NTB)]
        for sc in range(TC):
            sps = spsum.tile([P, TBLK], F32, tag="sps")
            for uc in range(UCH):
                nc.tensor.matmul(
                    sps,
                    lhsT=kT_sb[:, uc, ts(sc, P)],
                    rhs=qT_sb[:, uc, ds(tb * TBLK, TBLK)],
                    start=(uc == 0),
                    stop=(uc == UCH - 1),
                )
            pt = p_pool.tile([P, TBLK], BF16, tag="pt")
            nc.scalar.activation(out=pt, in_=sps, func=AF.Exp, scale=SCALE)
            pts.append(pt)
            if sc >= 2:
                pv_col(sc - 2, apss)
        pv_col(TC - 2, apss)
        pv_col(TC - 1, apss)
        # drain psum: recip of row sums + bf16 cast + TensorE transpose to aT
        rcps = []
        for tsl in range(NTB):
            aps = apss[tsl]
            rcp = rcp_pool.tile([P, 1], F32, tag="rcp")
            nc.vector.reciprocal(rcp, aps[:, U:U + 1])
            rcps.append(rcp)
            a_bf = a_pool.tile([P, U], BF16, tag="abf")
            nc.vector.tensor_copy(out=a_bf, in_=aps[:, 0:U])
            for uc in range(UCH):
                tps = spsum.tile([P, P], BF16, tag="sps")
                nc.tensor.transpose(tps, a_bf[:, ts(uc, P)], identity)
                nc.vector.tensor_copy(
                    out=aT_sb[:, uc, ds(tb * TBLK + tsl * P, P)], in_=tps,
                )
        # deferred output projection of the previous block (hides aT latency)
        if deferred[0] is not None:
            finish(*deferred[0])
        deferred[0] = (tb, rcps)
    finish(*deferred[0])

    for pool in (y_pool, rcp_pool, a_pool, p_pool,
                 ypsum, apsum, spsum, persist, consts):
        pool.release()


def _get_nc():
    if "nc" not in _cache:
        nc = bacc.Bacc("TRN2", target_bir_lowering=False, debug=False)
        with tile.TileContext(nc) as tc:
            _build_kernel(tc)
        nc.compile()
        _cache["nc"] = nc
    return _cache["nc"]


def kernel(**inputs):
    nc = _get_nc()
    shared = {k: np.ascontiguousarray(np.asarray(v, dtype=np.float32))
              for k, v in inputs.items() if k != "x"}
    xs = np.ascontiguousarray(np.asarray(inputs["x"], dtype=np.float32))
    in_maps = [dict(shared, x=xs[b]) for b in range(B)]
    res = run_bass_kernel_spmd(nc, in_maps, core_ids=list(range(B)))
    return np.stack([res.results[b]["out"] for b in range(B)], axis=0)



# revision 2
# speedup vs baseline: 1.3132x; 1.3132x over previous
"""Self-attention kernel for TRN2, data-parallel over batch (8 cores), fp8.

Per core (one batch element x[2048, 512]):
  - x loaded fp32 (residual), cast bf16 on ScalarE, transposed on TensorE
    to xT (c on partitions), stored fp8.
  - q/k/v projections and all attention matmuls run fp8 with
    perf_mode=DoubleRow (contraction pairs of 128-chunks -> ~2x TensorE).
  - scores computed TRANSPOSED (sT[s,t]) so the exp output feeds PV
    directly; exp = e^{score/16 - 2} (bias cancels in normalization),
    fused over two PSUM banks per activation.
  - row sums via a ones-column in v (free); reciprocal folded into the
    bf16 cast of a (per-partition scalar), so the output projection needs
    no further scaling.
  - biases: bq/bk exact via DVE per-partition add; bv/ba folded on the
    HOST into bc = Wa^T bv + ba, which the device adds into the fp32
    residual x once per tile (exact: attention rows sum to 1).

Matmul inputs fp8e4, PSUM accumulation fp32, softmax/normalize/residual fp32.
"""

import numpy as np

import concourse.bass as bass
import concourse.mybir as mybir
import concourse.tile as tile
from concourse import bacc
from concourse.bass import ds, ts
from concourse.bass_utils import run_bass_kernel_spmd
from concourse.masks import make_identity

F32 = mybir.dt.float32
BF16 = mybir.dt.bfloat16
F8 = mybir.dt.float8e4
AF = mybir.ActivationFunctionType
DR = mybir.MatmulPerfMode.DoubleRow

B, T, C, U, P = 8, 2048, 512, 256, 128
TC = T // P    # 16 row tiles
CCH = C // P   # 4 c-chunks
UCH = U // P   # 2 u-chunks
TBLK = 512     # t-block for attention
NTB = T // TBLK  # 4
VF = U + 16    # v free dim padded so the pair-dim stride is 16B-aligned
SCALE = 1.0 / float(np.sqrt(U))
EXPB = -2.0    # exp bias; cancels in row-sum normalization

_cache = {}


def _build_kernel(tc):
    nc = tc.nc
    x = nc.dram_tensor("x", [T, C], F32, kind="ExternalInput").ap()
    Wq = nc.dram_tensor("Wq", [C, U], F32, kind="ExternalInput").ap()
    Wk = nc.dram_tensor("Wk", [C, U], F32, kind="ExternalInput").ap()
    Wv = nc.dram_tensor("Wv", [C, U], F32, kind="ExternalInput").ap()
    Wa = nc.dram_tensor("Wa", [U, C], F32, kind="ExternalInput").ap()
    bq = nc.dram_tensor("bq", [U], F32, kind="ExternalInput").ap()
    bk = nc.dram_tensor("bk", [U], F32, kind="ExternalInput").ap()
    bcrep = nc.dram_tensor("bcrep", [P, C], F32, kind="ExternalInput").ap()
    out = nc.dram_tensor("out", [T, C], F32, kind="ExternalOutput").ap()

    consts = tc.alloc_tile_pool(name="consts", bufs=1)
    persist = tc.alloc_tile_pool(name="persist", bufs=1)

    identity = consts.tile([P, P], BF16)
    make_identity(nc, identity)

    # warm the ACT exp table early (one-time ~2.7us table load)
    dex = consts.tile([P, 1], F32)
    nc.vector.memset(dex, 0.0)
    expb = consts.tile([P, 1], F32)
    nc.vector.memset(expb, EXPB)
    dex2 = consts.tile([P, 1], F32)
    nc.scalar.activation(out=dex2, in_=dex, func=AF.Exp, bias=dex[:, 0:1],
                         scale=1.0)

    # persistent tensors
    x_sb = persist.tile([P, TC, C], F32)      # x rows (+bc), fp32 residual
    xT_f8 = persist.tile([P, CCH, T], F8)     # x^T  (c on partitions)
    qT_f8 = persist.tile([P, UCH, T], F8)     # q^T  (u on partitions)
    kT_f8 = persist.tile([P, UCH, T], F8)     # k^T
    v_sb = persist.tile([P, TC, VF], F8)      # v rows + ones col + pad
    aT_f8 = persist.tile([P, UCH, T], F8)     # a^T (normalized)
    nc.vector.memset(v_sb[:, :, U:VF], 0.0)
    nc.vector.memset(v_sb[:, :, U:U + 1], 1.0)

    # x loads: one DMA per row tile, alternating HWDGE queues
    for tt in range(TC):
        eng = nc.sync if tt % 2 == 0 else nc.scalar
        eng.dma_start(out=x_sb[:, tt, :], in_=x[ts(tt, P), :])

    # weights (fp32 HWDGE after x on the queues) + on-chip cast to fp8
    Wq_f8 = consts.tile([P, CCH, U], F8)
    Wk_f8 = consts.tile([P, CCH, U], F8)
    Wv_f8 = consts.tile([P, CCH, U], F8)
    Wa_f8 = consts.tile([P, UCH, C], F8)
    with tc.tile_pool(name="wstage", bufs=2) as wstage:
        for src, dst, pat, shp in (
            (Wq, Wq_f8, "(cc p) u -> p cc u", [P, CCH, U]),
            (Wk, Wk_f8, "(cc p) u -> p cc u", [P, CCH, U]),
            (Wv, Wv_f8, "(cc p) u -> p cc u", [P, CCH, U]),
            (Wa, Wa_f8, "(uc p) c -> p uc c", [P, UCH, C]),
        ):
            stage = wstage.tile(shp, F32, tag="wstg", name="stage")
            nc.sync.dma_start(out=stage, in_=src.rearrange(pat, p=P))
            nc.vector.tensor_copy(out=dst, in_=stage)
        bq_sb = consts.tile([P, UCH], F32)
        nc.scalar.dma_start(out=bq_sb, in_=bq.rearrange("(uc p) -> p uc", p=P))
        bk_sb = consts.tile([P, UCH], F32)
        nc.scalar.dma_start(out=bk_sb, in_=bk.rearrange("(uc p) -> p uc", p=P))
        bc_sb = consts.tile([P, C], F32)
        nc.scalar.dma_start(out=bc_sb, in_=bcrep)

        # PE warmup during the x DMA (HAM un-throttle needs ~4us busy)
        with tc.tile_pool(name="warm", bufs=1, space="PSUM") as warm_pool:
            wtile = warm_pool.tile([P, P], F32, name="warmup")
            for i in range(36):
                nc.tensor.matmul(wtile, lhsT=identity, rhs=identity,
                                 start=(i == 0), stop=(i == 35))

        # --- phase 1: xT via ACT cast + TensorE transpose; fold bc into x ---
        with tc.tile_pool(name="xbf", bufs=3) as xbf_pool, \
             tc.tile_pool(name="tpsum", bufs=2, space="PSUM") as tpsum:
            for tt in range(TC):
                x_bf = xbf_pool.tile([P, C], BF16, tag="xbf")
                nc.scalar.copy(out=x_bf, in_=x_sb[:, tt, :])
                tps = tpsum.tile([P, CCH, P], BF16, tag="tps")
                for cc in range(CCH):
                    nc.tensor.matmul(
                        tps[:, cc, :], lhsT=x_bf[:, ts(cc, P)], rhs=identity,
                        is_transpose=True, start=(cc == 0), stop=(cc == CCH - 1),
                    )
                nc.vector.tensor_copy(out=xT_f8[:, :, ts(tt, P)], in_=tps)
                nc.gpsimd.tensor_add(
                    out=x_sb[:, tt, :], in0=x_sb[:, tt, :], in1=bc_sb
                )

            # --- phase 2: projections (fp8 DoubleRow), grouped by t-block ---
            with tc.tile_pool(name="wpsum", bufs=2, space="PSUM") as wpsum, \
                 tc.tile_pool(name="vpsum", bufs=2, space="PSUM") as vpsum:
                for tb in range(NTB):
                    for tt in range(tb * 4, tb * 4 + 4):
                        vps = vpsum.tile([P, U], F32, tag="vps")
                        for i in range(2):
                            nc.tensor.matmul(
                                vps,
                                lhsT=xT_f8[:, 2 * i:2 * i + 2, ts(tt, P)],
                                rhs=Wv_f8[:, 2 * i:2 * i + 2, :],
                                start=(i == 0), stop=(i == 1), perf_mode=DR,
                            )
                        nc.vector.tensor_copy(out=v_sb[:, tt, 0:U], in_=vps)
                    for (W_f8, bias_sb, dst) in (
                        (Wk_f8, bk_sb, kT_f8), (Wq_f8, bq_sb, qT_f8),
                    ):
                        for uc in range(UCH):
                            wps = wpsum.tile([P, TBLK], F32, tag="wps")
                            for i in range(2):
                                nc.tensor.matmul(
                                    wps,
                                    lhsT=W_f8[:, 2 * i:2 * i + 2, ts(uc, P)],
                                    rhs=xT_f8[:, 2 * i:2 * i + 2, ds(tb * TBLK, TBLK)],
                                    start=(i == 0), stop=(i == 1), perf_mode=DR,
                                )
                            nc.vector.tensor_scalar(
                                out=dst[:, uc, ds(tb * TBLK, TBLK)], in0=wps,
                                scalar1=bias_sb[:, uc:uc + 1], scalar2=None,
                                op0=mybir.AluOpType.add,
                            )

    # --- phase 3: attention, fp8 DoubleRow, software-pipelined over tb ---
    spsum = tc.alloc_tile_pool(name="spsum", bufs=2, space="PSUM")
    apsum = tc.alloc_tile_pool(name="apsum", bufs=2, space="PSUM")
    p_pool = tc.alloc_tile_pool(name="p_pool", bufs=10)
    abf_pool = tc.alloc_tile_pool(name="abf_pool", bufs=6)
    rcp_pool = tc.alloc_tile_pool(name="rcp_pool", bufs=3)
    y_pool = tc.alloc_tile_pool(name="y_pool", bufs=3)

    def norm_cast(apss, abfs, tsl):
        """rcp of row sum, then a_bf = aps * rcp (normalized), fp32->bf16."""
        aps = apss[tsl]
        rcp = rcp_pool.tile([P, 1], F32, tag="rcp")
        nc.vector.reciprocal(rcp, aps[:, U:U + 1])
        a_bf = abf_pool.tile([P, U], BF16, tag="abf")
        nc.vector.tensor_scalar(
            out=a_bf, in0=aps[:, 0:U], scalar1=rcp, scalar2=None,
            op0=mybir.AluOpType.mult,
        )
        abfs[tsl] = a_bf

    def deferred_work(tb, abfs):
        """Transposes of a (tb) then output projection + residual (tb)."""
        chunks = []
        for tsl in range(NTB):
            def tchunk(tsl=tsl, tb=tb, abfs=abfs):
                row0 = tb * TBLK + tsl * P
                atps = apsum.tile([P, UCH, P], BF16, tag="misc", name="atps")
                for uc in range(UCH):
                    nc.tensor.matmul(
                        atps[:, uc, :], lhsT=abfs[tsl][:, ts(uc, P)],
                        rhs=identity, is_transpose=True,
                        start=(uc == 0), stop=(uc == UCH - 1),
                    )
                nc.vector.tensor_copy(out=aT_f8[:, :, ds(row0, P)], in_=atps)
            chunks.append(tchunk)
        for tsl in range(NTB):
            def fchunk(tsl=tsl, tb=tb):
                row0 = tb * TBLK + tsl * P
                yps = apsum.tile([P, TBLK], F32, tag="misc", name="yps")
                nc.tensor.matmul(
                    yps, lhsT=aT_f8[:, :, ds(row0, P)], rhs=Wa_f8[:, :, :],
                    start=True, stop=True, perf_mode=DR,
                )
                y_sb = y_pool.tile([P, C], F32, tag="ysb")
                nc.vector.tensor_add(
                    out=y_sb, in0=yps, in1=x_sb[:, tb * NTB + tsl, :]
                )
                nc.sync.dma_start(out=out[ds(row0, P), :], in_=y_sb)
            chunks.append(fchunk)
        return chunks

    deferred = []
    for tb in range(NTB):
        pts = []
        apss = [None] * NTB
        abfs = [None] * NTB
        for tsl in (0, 1):
            apss[tsl] = apsum.tile([P, VF], F32, tag="acc", name="apsA")
        todo = list(deferred)  # deferred chunks from tb-1
        for scp in range(8):
            sps = spsum.tile([P, 2, TBLK], F32, tag="sps")
            for j in range(2):
                nc.tensor.matmul(
                    sps[:, j, :],
                    lhsT=kT_f8[:, :, ts(2 * scp + j, P)],
                    rhs=qT_f8[:, :, ds(tb * TBLK, TBLK)],
                    start=True, stop=True, perf_mode=DR,
                )
            pt = p_pool.tile([P, 2, TBLK], F8, tag="pt")
            nc.scalar.activation(out=pt, in_=sps, func=AF.Exp,
                                 bias=expb[:, 0:1], scale=SCALE)
            pts.append(pt)
            # PV sweep A (tsl 0,1), one pair behind the exp
            if scp >= 1:
                for tsl in (0, 1):
                    nc.tensor.matmul(
                        apss[tsl],
                        lhsT=pts[scp - 1][:, :, ts(tsl, P)],
                        rhs=v_sb[:, 2 * (scp - 1):2 * scp, :],
                        start=(scp == 1), stop=False, perf_mode=DR,
                    )
            # interleave deferred transposes/output-proj of tb-1 into the
            # streak: they are dependency-ready and fill ACT-wait bubbles
            if scp >= 2:
                while todo and len(todo) > (7 - scp):
                    todo.pop(0)()
        for tsl in (0, 1):
            nc.tensor.matmul(
                apss[tsl], lhsT=pts[7][:, :, ts(tsl, P)],
                rhs=v_sb[:, 14:16, :], start=False, stop=True, perf_mode=DR,
            )
        while todo:
            todo.pop(0)()
        norm_cast(apss, abfs, 0)
        norm_cast(apss, abfs, 1)
        # PV sweep B (tsl 2,3) over the retained p tiles
        for tsl in (2, 3):
            apss[tsl] = apsum.tile([P, VF], F32, tag="acc", name="apsB")
        for scp in range(8):
            for tsl in (2, 3):
                nc.tensor.matmul(
                    apss[tsl],
                    lhsT=pts[scp][:, :, ts(tsl, P)],
                    rhs=v_sb[:, 2 * scp:2 * scp + 2, :],
                    start=(scp == 0), stop=(scp == 7), perf_mode=DR,
                )
        norm_cast(apss, abfs, 2)
        norm_cast(apss, abfs, 3)
        deferred = deferred_work(tb, abfs)
    for chunk in deferred:
        chunk()

    for pool in (y_pool, rcp_pool, abf_pool, p_pool,
                 apsum, spsum, persist, consts):
        pool.release()


def _get_nc():
    if "nc" not in _cache:
        nc = bacc.Bacc("TRN2", target_bir_lowering=False, debug=False)
        with tile.TileContext(nc) as tc:
            _build_kernel(tc)
        nc.compile()
        _cache["nc"] = nc
    return _cache["nc"]


def _host_inputs(inputs):
    f32 = np.float32
    Wa = np.ascontiguousarray(np.asarray(inputs["Wa"], dtype=f32))
    bc = np.asarray(inputs["bv"], dtype=f32) @ Wa + np.asarray(
        inputs["ba"], dtype=f32
    )
    bcrep = np.ascontiguousarray(
        np.broadcast_to(bc[None, :], (P, C)), dtype=f32
    )
    shared = {
        "Wq": np.ascontiguousarray(np.asarray(inputs["Wq"], dtype=f32)),
        "Wk": np.ascontiguousarray(np.asarray(inputs["Wk"], dtype=f32)),
        "Wv": np.ascontiguousarray(np.asarray(inputs["Wv"], dtype=f32)),
        "Wa": Wa,
        "bq": np.ascontiguousarray(np.asarray(inputs["bq"], dtype=f32)),
        "bk": np.ascontiguousarray(np.asarray(inputs["bk"], dtype=f32)),
        "bcrep": bcrep,
    }
    xs = np.ascontiguousarray(np.asarray(inputs["x"], dtype=f32))
    return [dict(shared, x=xs[b]) for b in range(B)]


def kernel(**inputs):
    nc = _get_nc()
    in_maps = _host_inputs(inputs)
    res = run_bass_kernel_spmd(nc, in_maps, core_ids=list(range(B)))
    return np.stack([res.results[b]["out"] for b in range(B)], axis=0)
